# revision 14
# baseline (speedup 1.0000x reference)
"""Trainium2 Bass kernel v7 for DirectedGraphConv.

Math (per batch b, node n):
    out = feature + einsum("bni,doi->bno", feature, weights) + bias[graph].sum(axis=2)

Identities:
  * einsum sums over directions d and input dim i:  out_mm = F @ (W0+W1+I)^T
    (+feature folds in as +I, added to the direction-sum on device).
  * bias[graph].sum(axis=2) = Count @ bias.  Counts come from 16 histogram
    matmuls over 16 linearly-independent "plane" functions of the labels:
    11 is_equal indicators (DVE), 4 ReLU ramps relu(g - a), a=11.5..14.5
    (ACT), and an all-ones plane (memset).  The change of basis back to
    per-label counts is an exact small-integer matrix A folded into the
    matmul selector stationaries (built on-chip).

v7 changes vs v6:
  * All bf16 inputs ride ONE packed HBM tensor, per-partition layout
    [cb4 | ftc0 | wtc0 | ftc1 | wtc1 | ftc2 | wtc2 | ftc3 | wtc3], moved by
    4 DMAs on the sync ring (g first as its own small u8 DMA).  This cuts
    HWDGE issue serialization (~0.7us/DMA) and lands bytes in dependency
    order.
  * Histogram matmuls are 4x column-tiled (M=16 output rows only): 4 planes
    run concurrently in disjoint 32-partition PSUM strips.  The strip
    reduction is folded into the bias matmul: cb4 replicates bias into all
    four strips (host-padded with zeros), so lhsT contraction over 128
    partitions sums the strips for free.
  * Bias matmul + final mains chunk interleave per output bank so banks
    close progressively; per-bank output DMAs split across both rings.

Device does all arithmetic.  Host only reshapes/transposes/casts/pads
inputs and upcasts the bf16 output.
"""

import numpy as np
import ml_dtypes

BF16 = ml_dtypes.bfloat16

B, N, D = 32, 128, 512
DIR = 2
L = 16
NC = 8
BPC = B // NC  # 4
BN = BPC * N  # 512
P = 128
KC = D // P  # 4
NDELTA = 11  # is_equal planes (labels 0..10)
NRAMP = 4  # relu ramp planes
FILLERS = 5

# packed bf16 tensor field offsets (in bf16 elements, per partition)
CB4_OFF = 0
CB4_LEN = D  # 512
CHUNK_LEN = BN + DIR * D  # ftc (512) + wtc (1024) = 1536
PK_LEN = CB4_LEN + KC * CHUNK_LEN  # 6656


def _ft_off(c):
    return CB4_LEN + c * CHUNK_LEN


def _wt_off(c, d):
    return CB4_LEN + c * CHUNK_LEN + BN + d * D


# DMA split points (bf16 elements): pk0 = [cb4|ftc0|wtc0], pk1..3 = [ftc|wtc]
_PK_SPLITS = [0, CB4_LEN + CHUNK_LEN, CB4_LEN + 2 * CHUNK_LEN,
              CB4_LEN + 3 * CHUNK_LEN, PK_LEN]

# plane ids: 0..10 = delta_l, 11..14 = ramp(11.5+i), 15 = ones
# cnt matmul firing order (by expected plane readiness: ones via memset,
# d0-d7 on DVE, ramps on ACT, d8-d10 on gpsimd); strip = index % 4
CNT_ORDER = [15, 0, 1, 2, 3, 4, 5, 6, 7, 11, 12, 13, 14, 8, 9, 10]


def _amatrix():
    xs = np.arange(L)
    planes = [(xs == l).astype(np.float64) for l in range(NDELTA)]
    for i in range(NRAMP):
        planes.append(np.maximum(xs - (NDELTA + 0.5 + i), 0.0))
    planes.append(np.ones(L))
    M = np.stack(planes)  # [16 planes, 16 labels]
    A = np.linalg.inv(M)  # counts = A @ S
    assert np.abs(A - np.round(A)).max() < 1e-9
    return np.round(A)  # A[j, k]: weight of plane k into count row j


_prog_cache: dict = {}


def _build():
    import concourse.bass as bass  # noqa: F401
    import concourse.mybir as mybir
    import concourse.tile as tile
    from concourse import bacc
    from concourse.masks import make_identity

    f32 = mybir.dt.float32
    bf16 = mybir.dt.bfloat16
    u8 = mybir.dt.uint8

    nc = bacc.Bacc("TRN2", target_bir_lowering=False, debug=False, num_devices=NC)

    g = nc.dram_tensor("g", [P, BN], u8, kind="ExternalInput").ap()
    pk = nc.dram_tensor("pk", [P, PK_LEN], bf16, kind="ExternalInput").ap()
    out = nc.dram_tensor("out", [BPC, N, D], bf16, kind="ExternalOutput").ap()

    A = _amatrix()

    with tile.TileContext(nc) as tc:
        with (
            tc.tile_pool(name="work", bufs=1) as wpool,
            tc.tile_pool(name="psum", bufs=1, space="PSUM") as ppool,
        ):
            # ---- DMAs first: g (tiny, feeds the longest chain) on the sync
            # ring; packed stream split across both HWDGE rings so the two
            # queues stream concurrently ----
            G_sb = wpool.tile([P, BN], u8)
            nc.sync.dma_start(out=G_sb, in_=g)
            pk_sb = wpool.tile([P, PK_LEN], bf16)
            for i, eng in zip(range(4), (nc.sync, nc.scalar, nc.sync, nc.scalar)):
                s, e = _PK_SPLITS[i], _PK_SPLITS[i + 1]
                eng.dma_start(out=pk_sb[:, s:e], in_=pk[:, s:e])

            # ---- on-chip constants (gpsimd) ----
            # esel[m, k, j] = A[j, k] for j < 16, 0 for j in 16..31.
            esel = wpool.tile([P, L, 32], bf16)
            nc.gpsimd.memset(esel, 0.0)
            esel_d = esel[:, 0:NDELTA, 0:L]
            # fill 1.0 on the delta diagonal (j == k)
            nc.gpsimd.affine_select(
                out=esel_d,
                in_=esel_d,
                compare_op=mybir.AluOpType.not_equal,
                fill=1.0,
                base=0,
                pattern=[[1, NDELTA], [-1, L]],
                channel_multiplier=0,
            )
            # delta planes also contribute -1 to count row 11
            nc.gpsimd.affine_select(
                out=esel_d,
                in_=esel_d,
                compare_op=mybir.AluOpType.not_equal,
                fill=-1.0,
                base=-NDELTA,
                pattern=[[0, NDELTA], [1, L]],
                channel_multiplier=0,
            )
            # ones plane column (needed by round 0)
            nc.gpsimd.memset(esel[:, 15, 11:12], float(A[11, 15]))
            # ones plane data + relu bias constants + identity
            planes = wpool.tile([P, L, BN], bf16)
            nc.gpsimd.memset(planes[:, 15, :], 1.0)
            rpb = wpool.tile([P, NRAMP], f32)
            for i in range(NRAMP):
                nc.gpsimd.memset(rpb[:, i : i + 1], -(NDELTA + 0.5 + i))
            ident = wpool.tile([P, P], bf16)
            make_identity(nc, ident)
            # ramp-plane selector columns (needed by round 3)
            for k in range(NDELTA, L - 1):
                for j in range(NDELTA, L):
                    v = float(A[j, k])
                    if v != 0.0:
                        nc.gpsimd.memset(esel[:, k, j : j + 1], v)

            # ---- ACT: table preload, then ramps as soon as G_bf exists ----
            act_warm = wpool.tile([P, 2], f32)
            nc.scalar.copy(out=act_warm[:, 0:1], in_=act_warm[:, 1:2])

            # ---- DVE stream ----
            dummy = wpool.tile([P, BN], bf16)
            nc.vector.memset(dummy, 1.0)
            G_bf = wpool.tile([P, BN], bf16)
            nc.vector.tensor_copy(out=G_bf, in_=G_sb)

            # ACT ramps (program order on scalar engine; wait on G_bf)
            for i in range(NRAMP):
                nc.scalar.activation(
                    out=planes[:, NDELTA + i, :],
                    in_=G_bf,
                    func=mybir.ActivationFunctionType.Relu,
                    bias=rpb[:, i : i + 1],
                    scale=1.0,
                )

            Wsum = wpool.tile([P, KC, D], bf16)

            def _wsum(c):
                nc.vector.tensor_tensor(
                    out=Wsum[:, c, :],
                    in0=pk_sb[:, _wt_off(c, 0) : _wt_off(c, 0) + D],
                    in1=pk_sb[:, _wt_off(c, 1) : _wt_off(c, 1) + D],
                    op=mybir.AluOpType.add,
                )
                sl = slice(c * P, (c + 1) * P)
                nc.vector.tensor_tensor(
                    out=Wsum[:, c, sl],
                    in0=Wsum[:, c, sl],
                    in1=ident,
                    op=mybir.AluOpType.add,
                )

            def _delta(l, eng):
                eng.tensor_scalar(
                    out=planes[:, l, :],
                    in0=G_bf,
                    scalar1=float(l),
                    scalar2=None,
                    op0=mybir.AluOpType.is_equal,
                )

            # DVE: deltas with Wsum chunk adds inserted by DMA arrival order:
            # pk1 (scalar ring head) ~= pk0 (sync head after g), then pk3, pk2
            # (gpsimd must NOT run elementwise here: its DVE-shared SBUF port
            # lock makes both engines ~25x slower — measured)
            for l in range(5):
                _delta(l, nc.vector)
            _wsum(1)
            for l in range(5, 8):
                _delta(l, nc.vector)
            _wsum(0)
            for l in range(8, 11):
                _delta(l, nc.vector)
            _wsum(3)
            _wsum(2)

            # ---- PE stream ----
            psum_warm = ppool.tile([P, BN], f32, tag="warm", bufs=1)
            psum_cnt = ppool.tile([P, BN], f32, tag="cnt", bufs=1)
            psum_outs = [
                ppool.tile([P, D], f32, tag=f"out{b}", bufs=1, name=f"psum_out{b}")
                for b in range(BPC)
            ]

            for _ in range(FILLERS):
                nc.tensor.matmul(
                    out=psum_warm, lhsT=dummy[:, 0:P], rhs=dummy,
                    start=True, stop=True,
                )

            def cnt_mm(i):
                k = CNT_ORDER[i]
                s = i % 4
                nc.tensor.matmul(
                    out=psum_cnt[32 * s : 32 * s + 32, :],
                    lhsT=esel[:, k, :],
                    rhs=planes[:, k, :],
                    start=i < 4,
                    stop=i >= 12,
                    tile_position=(0, 32 * s),
                )

            def mains(c, start=False, stop=False):
                fo = _ft_off(c)
                for b in range(BPC):
                    nc.tensor.matmul(
                        out=psum_outs[b],
                        lhsT=pk_sb[:, fo + b * P : fo + (b + 1) * P],
                        rhs=Wsum[:, c, :],
                        start=start,
                        stop=stop,
                    )

            for i in range(8):
                cnt_mm(i)
            mains(1, start=True)
            for i in range(8, 16):
                cnt_mm(i)
            mains(0)

            # per-bank slice copies so each bias matmul waits only its slice
            cntT = wpool.tile([P, BN], bf16)
            for b in range(BPC):
                nc.scalar.copy(
                    out=cntT[:, b * P : (b + 1) * P],
                    in_=psum_cnt[:, b * P : (b + 1) * P],
                )

            # bias matmuls, then mains c3, then mains c2 (last to land)
            # closing each output bank progressively
            cb4 = pk_sb[:, CB4_OFF : CB4_OFF + CB4_LEN]
            out_sb = wpool.tile([P, BPC, D], bf16)
            h = D // 2
            for b in range(BPC):
                nc.tensor.matmul(
                    out=psum_outs[b],
                    lhsT=cntT[:, b * P : (b + 1) * P],
                    rhs=cb4,
                    start=False,
                    stop=False,
                )
            mains(3)
            fo2 = _ft_off(2)
            for b in range(BPC):
                nc.tensor.matmul(
                    out=psum_outs[b],
                    lhsT=pk_sb[:, fo2 + b * P : fo2 + (b + 1) * P],
                    rhs=Wsum[:, 2, :],
                    start=False,
                    stop=True,
                )
                nc.vector.tensor_copy(out=out_sb[:, b, 0:h], in_=psum_outs[b][:, 0:h])
                nc.scalar.copy(out=out_sb[:, b, h:D], in_=psum_outs[b][:, h:D])
                eng = nc.sync if b % 2 == 0 else nc.scalar
                eng.dma_start(out=out[b], in_=out_sb[:, b, :])

    nc.compile()
    return nc


def _get_prog():
    if "v7" not in _prog_cache:
        _prog_cache["v7"] = _build()
    return _prog_cache["v7"]


def _shard_inputs(feature, graph, weights, bias):
    f = np.asarray(feature, dtype=np.float32)
    g8 = np.asarray(graph).astype(np.uint8)
    w = np.asarray(weights, dtype=np.float32)
    b16 = np.asarray(bias, dtype=np.float32).astype(BF16)

    # cb4[p] = bias[p % 32] if p % 32 < 16 else 0   (strip-replicated)
    cb4 = np.zeros((P, D), dtype=BF16)
    for s in range(4):
        cb4[32 * s : 32 * s + L] = b16

    # wt[p, c, d, o] = w[d, o, c*128+p]   (replicated across cores)
    wt = np.ascontiguousarray(
        w.transpose(2, 0, 1).reshape(KC, P, DIR, D).transpose(1, 0, 2, 3)
    ).astype(BF16)  # [p, c, d, o]

    in_maps = []
    for core in range(NC):
        sl = slice(core * BPC, (core + 1) * BPC)
        fc = f[sl]  # [BPC, N, D]
        ftc = np.ascontiguousarray(
            fc.transpose(2, 0, 1).reshape(KC, P, BN).transpose(1, 0, 2)
        ).astype(BF16)  # [p, c, bn]
        gc = np.ascontiguousarray(g8[sl].transpose(2, 0, 1).reshape(P, BN))
        pk = np.empty((P, PK_LEN), dtype=BF16)
        pk[:, CB4_OFF : CB4_OFF + CB4_LEN] = cb4
        for c in range(KC):
            pk[:, _ft_off(c) : _ft_off(c) + BN] = ftc[:, c, :]
            pk[:, _wt_off(c, 0) : _wt_off(c, 1) + D] = wt[:, c].reshape(P, DIR * D)
        in_maps.append({"g": gc, "pk": pk})
    return in_maps


def _run(feature, graph, weights, bias, trace=False):
    from concourse.bass_utils import run_bass_kernel_spmd

    in_maps = _shard_inputs(feature, graph, weights, bias)
    nc = _get_prog()
    res = run_bass_kernel_spmd(nc, in_maps, core_ids=list(range(NC)), trace=trace)
    out = np.concatenate(
        [r["out"].astype(np.float32) for r in res.results], axis=0
    )
    return out, res


def kernel(feature, graph, weights, bias):
    out, _ = _run(feature, graph, weights, bias, trace=False)
    return out
